# revision 7
# baseline (speedup 1.0000x reference)
"""Trainium2 Bass kernel for nn_CombiIntegral (spectral-shift + finite-diff CNN).

The reference computes, for x of shape (4, 32, 256, 256):
  - 9 "spectral" channel blocks: FFT-domain circular conv with one-hot 3x3
    kernels == circular rolls of x by (dy, dx) in {-1,0,1}^2.
  - 3 depthwise finite-difference convs (zero padding 1):
        d0 = x[h, w-1] - x[h, w]
        d1 = x[h-1, w] - x[h, w]
        d2 = x[h-1, w-1] - x[h, w]
  - output = concat of the 12 blocks along channels: (4, 384, 256, 256).

Strategy: pure data parallel over the 128 (batch, channel) planes -> 16 planes
per NeuronCore.  Per core: load the 4 MB input slice into SBUF once (widened
with 2 halo columns per row so every w-roll is a contiguous window), build a
one-row-down-shifted SBUF copy S (so the h-shifted diff operands stay
partition-aligned for the vector engine, which requires start partition in
{0,32,64,96}), then write each of the 12 output blocks straight from SBUF
with shifted access patterns (9 rolls = pure DMA; 3 diffs = DVE subtract into
result tiles, then DMA out).  Output DMAs are split across the SP and ACT
HWDGE rings.  HBM traffic per core ~= 4 MB read + 48 MB write.

`build_nc(repeats=R)` unrolls the whole sequence R times with a drain barrier
between repetitions - used by the timing harness to amortize dispatch
overhead (time(R2)-time(R1))/(R2-R1).  The graded kernel uses repeats=1.
"""

import numpy as np

import concourse.bass as bass
import concourse.mybir as mybir
from concourse.bass_utils import run_bass_kernel_spmd

F32 = mybir.dt.float32
NCORES = 8
P = 16        # planes per core
H = 256
W = 256
WP = W + 2    # T row pitch: [halo_left | w=0..255 | halo_right]
NB = 12       # output blocks

# roll segments: for a vertical shift dy, out rows [out_h0, out_h0+n) come from
# source rows [p0, p0+n) of tile `t` (t0 = x rows 0..127, t1 = x rows 128..255)
_ROLL_SEGS = {
    -1: [(0, 0, 1, 127), (127, 1, 0, 128), (255, 0, 0, 1)],
    0: [(0, 0, 0, 128), (128, 1, 0, 128)],
    1: [(1, 0, 0, 128), (129, 1, 0, 127), (0, 1, 127, 1)],
}

N_OUT_DMAS = 24 + 6   # per repetition


def build_nc(repeats: int = 1) -> bass.Bass:
    nc = bass.Bass()
    x = nc.dram_tensor("x", [P, H, W], F32, kind="ExternalInput")
    y = nc.dram_tensor("y", [NB, P, H, W], F32, kind="ExternalOutput")

    xh = x.rearrange("c h w -> h c w")                      # (256, 16, 256)
    yh = [y[i].rearrange("c h w -> h c w") for i in range(NB)]

    with (
        nc.sbuf_tensor([128, P * WP], F32) as t0,
        nc.sbuf_tensor([128, P * WP], F32) as t1,
        nc.sbuf_tensor([128, P * W], F32) as s0,
        nc.sbuf_tensor([128, P * W], F32) as s1,
        nc.sbuf_tensor([128, P * W], F32) as r00,
        nc.sbuf_tensor([128, P * W], F32) as r01,
        nc.sbuf_tensor([128, P * W], F32) as r10,
        nc.sbuf_tensor([128, P * W], F32) as r11,
        nc.sbuf_tensor([128, P * W], F32) as r20,
        nc.sbuf_tensor([128, P * W], F32) as r21,
        nc.semaphore() as main_sem,
        nc.semaphore() as halo_sem,
        nc.semaphore() as s_sem,
        nc.semaphore() as dve_sem,
        nc.semaphore() as out_sem,
        nc.Block() as block,
    ):
        T = [
            t0[:, :].rearrange("p (c q) -> p c q", c=P),    # [128, 16, 258]
            t1[:, :].rearrange("p (c q) -> p c q", c=P),
        ]
        # S[t][p][c][j] = x[c, 128*t + p - 1, j]; S[0][0] is zeroed (h pad)
        S = [
            s0[:, :].rearrange("p (c q) -> p c q", c=P),    # [128, 16, 256]
            s1[:, :].rearrange("p (c q) -> p c q", c=P),
        ]
        R = [
            [r00[:, :].rearrange("p (c q) -> p c q", c=P),
             r01[:, :].rearrange("p (c q) -> p c q", c=P)],
            [r10[:, :].rearrange("p (c q) -> p c q", c=P),
             r11[:, :].rearrange("p (c q) -> p c q", c=P)],
            [r20[:, :].rearrange("p (c q) -> p c q", c=P),
             r21[:, :].rearrange("p (c q) -> p c q", c=P)],
        ]
        halves = ((0, 0, 128), (1, 128, 256))

        def emit_roll(eng, i, base_out):
            r, c = divmod(i, 3)
            dy, dx = r - 1, c - 1
            q0 = 1 - dx  # SBUF col window start for this w-shift
            for out_h0, t, p0, n in _ROLL_SEGS[dy]:
                eng.dma_start(
                    out=yh[i][out_h0:out_h0 + n],
                    in_=T[t][p0:p0 + n, :, q0:q0 + W],
                ).then_inc(out_sem, 16)

        # per-repetition semaphore bases
        def bases(j):
            return dict(
                main=j * 32, halo=j * 64, s=j * 48,
                dve=1 + j * 6,          # +1 for the one-time memset
                out=j * 16 * N_OUT_DMAS,
            )

        @block.sync
        def _(sync):
            for j in range(repeats):
                b = bases(j)
                if j > 0:
                    # all previous outputs flushed => T/S/R free to reuse
                    sync.wait_ge(out_sem, b["out"])
                # main loads: x rows -> partitions, planes along free dim
                for t, h0, h1 in halves:
                    sync.dma_start(
                        out=T[t][:, :, 1:W + 1], in_=xh[h0:h1]
                    ).then_inc(main_sem, 16)
                # halo columns (circular in w within each row); 4 B elements
                # -> inherently scattered, but only ~8 KB per DMA
                with nc.allow_non_contiguous_dma(reason="halo cols, 8KB each"):
                    for t, h0, h1 in halves:
                        sync.dma_start(
                            out=T[t][:, :, 0:1], in_=xh[h0:h1, :, W - 1:W]
                        ).then_inc(halo_sem, 16)
                        sync.dma_start(
                            out=T[t][:, :, W + 1:W + 2], in_=xh[h0:h1, :, 0:1]
                        ).then_inc(halo_sem, 16)

                sync.wait_ge(main_sem, b["main"] + 32)
                for i in (1, 4):             # dx == 0: no halo needed
                    emit_roll(sync, i, b["out"])
                sync.wait_ge(halo_sem, b["halo"] + 64)
                for i in (0, 2, 3):
                    emit_roll(sync, i, b["out"])

                # diff outputs as DVE finishes each result tile
                for d in range(3):
                    for t, h0, h1 in halves:
                        sync.wait_ge(dve_sem, b["dve"] + 2 * d + t + 1)
                        sync.dma_start(
                            out=yh[9 + d][h0:h1], in_=R[d][t][:, :, :]
                        ).then_inc(out_sem, 16)

            sync.wait_ge(out_sem, repeats * 16 * N_OUT_DMAS)

        @block.scalar
        def _(scalar):
            for j in range(repeats):
                b = bases(j)
                if j > 0:
                    scalar.wait_ge(out_sem, b["out"])
                    # prior rep's DVE reads of S must be done before rewrite
                    scalar.wait_ge(dve_sem, b["dve"])
                scalar.wait_ge(main_sem, b["main"] + 32)
                # build S = x shifted down one row, via SBUF->SBUF partition-
                # shifted copies (no extra HBM traffic)
                scalar.dma_start(
                    out=S[0][1:128, :, :], in_=T[0][0:127, :, 1:W + 1]
                ).then_inc(s_sem, 16)
                scalar.dma_start(
                    out=S[1][0:1, :, :], in_=T[0][127:128, :, 1:W + 1]
                ).then_inc(s_sem, 16)
                scalar.dma_start(
                    out=S[1][1:128, :, :], in_=T[1][0:127, :, 1:W + 1]
                ).then_inc(s_sem, 16)
                emit_roll(scalar, 7, b["out"])      # dx == 0
                scalar.wait_ge(halo_sem, b["halo"] + 64)
                for i in (5, 6, 8):
                    emit_roll(scalar, i, b["out"])

        @block.vector
        def _(vector):
            v = nc.vector
            # zero the h=-1 row of S once (the S copies write partitions
            # 1..127 only - no overlap); the sem inc also orders the later
            # same-engine reads for the race model
            v.memset(S[0][0:1, :, :], 0.0).then_inc(dve_sem, 1)
            vector.wait_ge(dve_sem, 1)
            for j in range(repeats):
                b = bases(j)
                if j > 0:
                    vector.wait_ge(out_sem, b["out"])
                vector.wait_ge(main_sem, b["main"] + 32)
                # d0: out[h, w] = x[h, w-1] - x[h, w]  (zero pad in w)
                for t in (0, 1):
                    rr = R[0][t]
                    v.tensor_sub(
                        rr[:, :, 1:W], T[t][:, :, 1:W], T[t][:, :, 2:W + 1]
                    )
                    v.tensor_scalar_mul(
                        rr[:, :, 0:1], T[t][:, :, 1:2], -1.0
                    ).then_inc(dve_sem, 1)
                vector.wait_ge(s_sem, b["s"] + 48)
                # d1: out[h, w] = x[h-1, w] - x[h, w]  (zero pad in h)
                for t in (0, 1):
                    rr = R[1][t]
                    v.tensor_sub(
                        rr[:, :, :], S[t][:, :, :], T[t][:, :, 1:W + 1]
                    ).then_inc(dve_sem, 1)
                # d2: out[h, w] = x[h-1, w-1] - x[h, w]  (zero pad both)
                for t in (0, 1):
                    rr = R[2][t]
                    v.tensor_sub(
                        rr[:, :, 1:W], S[t][:, :, 0:W - 1], T[t][:, :, 2:W + 1]
                    )
                    v.tensor_scalar_mul(
                        rr[:, :, 0:1], T[t][:, :, 1:2], -1.0
                    ).then_inc(dve_sem, 1)

    return nc


_NC_CACHE = None


def _get_nc() -> bass.Bass:
    global _NC_CACHE
    if _NC_CACHE is None:
        _NC_CACHE = build_nc()
    return _NC_CACHE


def kernel(x: np.ndarray) -> np.ndarray:
    x = np.ascontiguousarray(np.asarray(x), dtype=np.float32)
    B, C, _, _ = x.shape                         # (4, 32, 256, 256)
    planes = x.reshape(NCORES, P, H, W)          # plane p = b*C + c, core k = p // 16
    in_maps = [{"x": planes[k]} for k in range(NCORES)]
    res = run_bass_kernel_spmd(_get_nc(), in_maps, list(range(NCORES)))
    outs = np.stack([res.results[k]["y"] for k in range(NCORES)])  # (8, 12, 16, H, W)
    out = (
        outs.transpose(1, 0, 2, 3, 4)            # (12, 8, 16, H, W)
        .reshape(NB, B, C, H, W)
        .transpose(1, 0, 2, 3, 4)                # (4, 12, 32, H, W)
        .reshape(B, NB * C, H, W)
    )
    return out
